# revision 1
# baseline (speedup 1.0000x reference)
"""Multi-head cross-attention (B=2, N=1024, L=4096, D=1024, H=16) on 8 trn2
NeuronCores.

Sharding: batch x head-group data/tensor parallel. Core c handles batch
c//4 and heads 4*(c%4) .. 4*(c%4)+3 (weight columns sliced per head group,
Wo row-sliced; partial outputs summed on the host during unsharding).

Per-core device program (all matmuls in fp32r at full PE rate):
  qT/kT = W.T @ x.T    (channels on partitions, head pairs stacked 64+64)
  v     = x @ Wv       (keys on partitions) augmented with a ones column and
                       pre-multiplied by the pad-keep mask (this implements
                       the padding mask exactly: masked keys contribute to
                       neither numerator nor denominator)
  per (query-block, head-pair, keytile):
     sT[keys,q] = kT.T @ qT   (two row-paired K=64 matmuls)
     pT = exp(0.125 * sT)     (one ACT op over both heads' banks)
     oT_aug[65,q] += v_aug.T @ pT   (PSUM accumulation; row 64 = denominator)
  out_part = (oT/denom).T @ Wo_slice   (+ q/k/v biases via K=1 matmuls)
"""
import sys

sys.path.insert(0, "/opt/trn_rl_repo")

import numpy as np

import concourse.bass as bass
import concourse.tile as tile
from concourse import bacc, mybir
from concourse.bass_utils import run_bass_kernel_spmd

dt = mybir.dt
ts = bass.ts

B, N, L, D = 2, 1024, 4096, 1024
H, DH = 16, 64
HC = 4            # heads per core
CS = HC * DH      # 256 channel slice per core
SCALE = DH ** -0.5
N_CORES = 8
QB, KB = 2, 8     # query blocks of 512, key blocks of 512
DQC = 8           # contraction chunks of 128
KT = 32           # keytiles of 128

TRACE = False
LAST_EXEC_NS = None
_cache = {}


def _build():
    nc = bacc.Bacc("TRN2", target_bir_lowering=False, debug=False,
                   num_devices=N_CORES)

    xTq = nc.dram_tensor("xTq", [D, N], dt.float32, kind="ExternalInput").ap()
    xTkv = nc.dram_tensor("xTkv", [D, L], dt.float32, kind="ExternalInput").ap()
    wq = nc.dram_tensor("wq", [D, CS], dt.float32, kind="ExternalInput").ap()
    wk = nc.dram_tensor("wk", [D, CS], dt.float32, kind="ExternalInput").ap()
    wv = nc.dram_tensor("wv", [D, CS], dt.float32, kind="ExternalInput").ap()
    wo = nc.dram_tensor("wo", [CS, D], dt.float32, kind="ExternalInput").ap()
    bqv = nc.dram_tensor("bqv", [1, CS], dt.float32, kind="ExternalInput").ap()
    bkv = nc.dram_tensor("bkv", [1, CS], dt.float32, kind="ExternalInput").ap()
    bvv = nc.dram_tensor("bvv", [1, CS], dt.float32, kind="ExternalInput").ap()
    keep = nc.dram_tensor("keep", [128, KT, HC], dt.float32,
                          kind="ExternalInput").ap()
    out = nc.dram_tensor("out", [N, D], dt.float32, kind="ExternalOutput").ap()

    with tile.TileContext(nc) as tc:
        _emit(nc, tc, xTq, xTkv, wq, wk, wv, wo, bqv, bkv, bvv, keep, out)
    nc.compile()
    return nc


def _emit(nc, tc, xTq, xTkv, wq, wk, wv, wo, bqv, bkv, bvv, keep, out):
    import contextlib

    ctx = contextlib.ExitStack()
    with ctx:
        persist = ctx.enter_context(tc.tile_pool(name="persist", bufs=1))
        wstage = ctx.enter_context(tc.tile_pool(name="wstage", bufs=2))
        xstage = ctx.enter_context(tc.tile_pool(name="xstage", bufs=3))
        xr_pool = ctx.enter_context(tc.tile_pool(name="xr", bufs=10))
        pT_pool = ctx.enter_context(tc.tile_pool(name="pT", bufs=3))
        rb_pool = ctx.enter_context(tc.tile_pool(name="rbs", bufs=2))
        outsb_pool = ctx.enter_context(tc.tile_pool(name="outsb", bufs=3))
        psS = ctx.enter_context(tc.tile_pool(name="psS", bufs=2, space="PSUM"))
        psOA = ctx.enter_context(tc.tile_pool(name="psOA", bufs=1, space="PSUM"))
        psA_cm = tc.tile_pool(name="psA", bufs=1, space="PSUM")
        psA = psA_cm.__enter__()
        lp = nc.allow_low_precision(reason="fp32r attention internals")
        lp.__enter__()

        # ---- weight loading: one big DMA + one cast each -----------------
        def load_w3(name, src, d0):
            # src: DRAM [d0*128, F]; dst tile [128, d0, F] (chunk-major)
            f = wstage.tile([128, d0, src.shape[1]], dt.float32, tag="wstage",
                            name=f"{name}_f")
            nc.sync.dma_start(f[:], src.rearrange("(c p) n -> p c n", p=128))
            r = persist.tile([128, d0, src.shape[1]], dt.float32r, tag=name,
                             name=name)
            nc.vector.tensor_copy(r[:], f[:])
            return r

        def load_round(name, src, shape):
            f = wstage.tile(shape, dt.float32, tag="bstage", name=f"{name}_f")
            nc.sync.dma_start(f[:], src)
            r = persist.tile(shape, dt.float32r, tag=name, name=name)
            nc.vector.tensor_copy(r[:], f[:])
            return r

        wq_r = load_w3("wqr", wq, DQC)          # [128, 8, 256]
        bq_r = load_round("bqr", bqv, [1, CS])
        ones_f = wstage.tile([1, 512], dt.float32, tag="bstage", name="ones_f")
        nc.vector.memset(ones_f[:], 1.0)
        ones512_r = persist.tile([1, 512], dt.float32r, tag="o512", name="ones512_r")
        nc.vector.tensor_copy(ones512_r[:], ones_f[:])
        ones128_r = persist.tile([1, 128], dt.float32r, tag="o128", name="ones128_r")
        nc.vector.tensor_copy(ones128_r[:], ones_f[:, 0:128])

        # ---- persistent activation tiles --------------------------------
        qT_sb = [persist.tile([128, N], dt.float32r, tag=f"qT{cc}", name=f"qT{cc}")
                 for cc in range(2)]
        kT_sb = [[persist.tile([128, 512], dt.float32r, tag=f"kT{cc}_{kb}",
                               name=f"kT{cc}_{kb}") for kb in range(KB)]
                 for cc in range(2)]
        va_sb = [persist.tile([128, HC, 65], dt.float32r, tag=f"va{kt}",
                              name=f"va{kt}") for kt in range(KT)]
        onT_sb = [persist.tile([128, N], dt.float32r, tag=f"onT{cc}",
                               name=f"onT{cc}") for cc in range(2)]

        # ---- Q projection ----------------------------------------------
        for qb in range(QB):
            qp = psA.tile([128, 1024], dt.float32, tag="psA", name=f"qp{qb}")
            for dq in range(DQC):
                xf = xstage.tile([128, 512], dt.float32, tag="xs", name=f"xfq{qb}_{dq}")
                nc.sync.dma_start(xf[:], xTq[ts(dq, 128), ts(qb, 512)])
                xr = xr_pool.tile([128, 512], dt.float32r, tag="xr", name=f"xrq{qb}_{dq}")
                nc.vector.tensor_copy(xr[:], xf[:])
                for cc in range(2):
                    nc.tensor.matmul(qp[:, ts(cc, 512)], wq_r[:, dq, ts(cc, 128)],
                                     xr[:], start=(dq == 0), stop=False)
            for cc in range(2):
                nc.tensor.matmul(qp[:, ts(cc, 512)], bq_r[:, ts(cc, 128)],
                                 ones512_r[:], start=False, stop=True)
                nc.vector.tensor_copy(qT_sb[cc][:, ts(qb, 512)], qp[:, ts(cc, 512)])

        # remaining weights (DMA priority after the q-projection inputs)
        wk_r = load_w3("wkr", wk, DQC)
        wv_r = load_w3("wvr", wv, DQC)
        bk_r = load_round("bkr", bkv, [1, CS])
        bv_r = load_round("bvr", bvv, [1, CS])
        keep_f = persist.tile([128, KT, HC], dt.float32, tag="keepf", name="keep_f")
        nc.sync.dma_start(keep_f[:], keep)

        # ---- attention helpers ------------------------------------------
        oPs = {}

        def open_oP(qb, hp, pool, sfx):
            oPs[(qb, hp)] = [
                pool.tile([128, 512], dt.float32, tag=f"oP{h}{sfx}",
                          name=f"oP{qb}{hp}{h}")
                for h in range(2)
            ]

        def attn_kt(qb, hp, kt):
            kb, kti = kt // 4, kt % 4
            sp = psS.tile([128, 1024], dt.float32, tag="sp", name=f"sp{qb}{hp}{kt}")
            for h in range(2):
                nc.tensor.matmul(
                    sp[:, ts(h, 512)],
                    kT_sb[hp][kb][ts(h, 64), ts(kti, 128)],
                    qT_sb[hp][ts(h, 64), ts(qb, 512)],
                    start=True, stop=True,
                )
            pT = pT_pool.tile([128, 1024], dt.float32r, tag="pT", name=f"pT{qb}{hp}{kt}")
            nc.scalar.activation(pT[:], sp[:], mybir.ActivationFunctionType.Exp,
                                 scale=float(SCALE))
            oP = oPs[(qb, hp)]
            for h in range(2):
                nc.tensor.matmul(
                    oP[h][0:65, :], va_sb[kt][:, hp * 2 + h, :], pT[:, ts(h, 512)],
                    start=(kt == 0), stop=(kt == KT - 1),
                )

        def attn_norm(qb, hp):
            oP = oPs.pop((qb, hp))
            for h in range(2):
                den = rb_pool.tile([1, 512], dt.float32, tag="den",
                                   name=f"den{qb}{hp}{h}")
                nc.vector.tensor_copy(den[:], oP[h][64:65, :])
                rdf = rb_pool.tile([1, 512], dt.float32, tag="rdf",
                                   name=f"rdf{qb}{hp}{h}")
                # approx_fast needs an SBUF source (PSUM source returns garbage)
                nc.vector.reciprocal_approx_fast(rdf[:], den[:])
                rd = rb_pool.tile([1, 512], dt.float32r, tag="rd",
                                  name=f"rd{qb}{hp}{h}")
                nc.vector.tensor_copy(rd[:], rdf[:])
                rb = psS.tile([128, 512], dt.float32, tag="sp", name=f"rb{qb}{hp}{h}")
                nc.tensor.matmul(rb[:, :], ones128_r[:], rd[:], start=True, stop=True)
                rb_sb = rb_pool.tile([128, 512], dt.float32, tag="rbs",
                                     name=f"rbs{qb}{hp}{h}")
                nc.vector.tensor_copy(rb_sb[:], rb[:])
                nc.vector.tensor_mul(onT_sb[hp][ts(h, 64), ts(qb, 512)],
                                     oP[h][0:64, :], rb_sb[0:64, :])

        # ---- K/V projections interleaved with attention on (qb0, hp0) ---
        open_oP(0, 0, psOA, "a")
        for kb in range(KB):
            kp = psA.tile([128, 1024], dt.float32, tag="psA", name=f"kp{kb}")
            xrs = []
            for dq in range(DQC):
                xf = xstage.tile([128, 512], dt.float32, tag="xs", name=f"xfk{kb}_{dq}")
                nc.sync.dma_start(xf[:], xTkv[ts(dq, 128), ts(kb, 512)])
                xr = xr_pool.tile([128, 512], dt.float32r, tag="xr", name=f"xrk{kb}_{dq}")
                nc.vector.tensor_copy(xr[:], xf[:])
                xrs.append(xr)
                for cc in range(2):
                    nc.tensor.matmul(kp[:, ts(cc, 512)], wk_r[:, dq, ts(cc, 128)],
                                     xr[:], start=(dq == 0), stop=False)
            for cc in range(2):
                nc.tensor.matmul(kp[:, ts(cc, 512)], bk_r[:, ts(cc, 128)],
                                 ones512_r[:], start=False, stop=True)
                nc.vector.tensor_copy(kT_sb[cc][kb][:], kp[:, ts(cc, 512)])

            vp = psA.tile([128, 1024], dt.float32, tag="psA", name=f"vp{kb}")
            for dq in range(DQC):
                for t in range(4):
                    # start clears has_written for the whole 2KB psum bank, so
                    # only the first matmul touching each bank may set it
                    nc.tensor.matmul(vp[:, ts(t, 256)], xrs[dq][:, ts(t, 128)],
                                     wv_r[:, dq, :],
                                     start=(dq == 0 and t % 2 == 0), stop=False)
            for t in range(4):
                nc.tensor.matmul(vp[:, ts(t, 256)], ones128_r[:], bv_r[:],
                                 start=False, stop=True)
            for t in range(4):
                kt = kb * 4 + t
                va = va_sb[kt]
                src = vp[:, ts(t, 256)].rearrange("p (h c) -> p h c", h=HC)
                nc.vector.tensor_scalar_mul(va[:, :, 0:64], src,
                                            keep_f[:, kt, 0:1])
                nc.vector.tensor_copy(va[:, :, 64:65], keep_f[:, kt, :])

            for t in range(4):
                attn_kt(0, 0, kb * 4 + t)

        # projections done: release psA's 2 banks, open the second oP pool
        psA_cm.__exit__(None, None, None)
        psOB = ctx.enter_context(tc.tile_pool(name="psOB", bufs=1, space="PSUM"))

        wo_r = load_w3("wor", wo, 2)            # [128, 2, 1024]

        attn_norm(0, 0)

        # ---- remaining attention combos (alternating psum pools) --------
        for i, (qb, hp) in enumerate([(0, 1), (1, 0), (1, 1)]):
            pool, sfx = (psOB, "b") if i % 2 == 0 else (psOA, "a")
            open_oP(qb, hp, pool, sfx)
            for kt in range(KT):
                attn_kt(qb, hp, kt)
            attn_norm(qb, hp)

        # ---- output projection ------------------------------------------
        for qt in range(8):
            for eb in range(2):
                pool, sfx = (psOB, "b") if (qt * 2 + eb) % 2 == 0 else (psOA, "a")
                op = pool.tile([128, 512], dt.float32, tag=f"oP0{sfx}",
                               name=f"op{qt}_{eb}")
                for cc in range(2):
                    nc.tensor.matmul(op[:, :], onT_sb[cc][:, ts(qt, 128)],
                                     wo_r[:, cc, ts(eb, 512)],
                                     start=(cc == 0), stop=(cc == 1))
                osb = outsb_pool.tile([128, 512], dt.float32, tag="osb",
                                      name=f"osb{qt}_{eb}")
                nc.vector.tensor_copy(osb[:], op[:])
                nc.sync.dma_start(out[ts(qt, 128), ts(eb, 512)], osb[:])

        lp.__exit__(None, None, None)


def kernel(x_q, x_kv, pad_mask, Wq, bq, Wk, bk, Wv, bv, Wo, bo):
    global LAST_EXEC_NS
    x_q = np.asarray(x_q, np.float32)
    x_kv = np.asarray(x_kv, np.float32)
    pad_mask = np.asarray(pad_mask)
    Wq, bq = np.asarray(Wq, np.float32), np.asarray(bq, np.float32)
    Wk, bk = np.asarray(Wk, np.float32), np.asarray(bk, np.float32)
    Wv, bv = np.asarray(Wv, np.float32), np.asarray(bv, np.float32)
    Wo, bo = np.asarray(Wo, np.float32), np.asarray(bo, np.float32)

    if "nc" not in _cache:
        _cache["nc"] = _build()
    nc = _cache["nc"]

    xTq = [np.ascontiguousarray(x_q[b].T) for b in range(B)]
    xTkv = [np.ascontiguousarray(x_kv[b].T) for b in range(B)]
    keepm = []
    for b in range(B):
        k01 = (~pad_mask[b]).astype(np.float32)          # (L,) 1=keep
        k4 = np.repeat(k01[:, None], HC, axis=1)          # (L, HC)
        keepm.append(np.ascontiguousarray(
            k4.reshape(KT, 128, HC).transpose(1, 0, 2)))  # (128, KT, HC)

    in_maps = []
    for c in range(N_CORES):
        b, g = c // 4, c % 4
        hs = g * CS
        in_maps.append({
            "xTq": xTq[b],
            "xTkv": xTkv[b],
            "wq": np.ascontiguousarray(Wq[:, hs:hs + CS]),
            "wk": np.ascontiguousarray(Wk[:, hs:hs + CS]),
            "wv": np.ascontiguousarray(Wv[:, hs:hs + CS]),
            "wo": np.ascontiguousarray(Wo[hs:hs + CS, :]),
            "bqv": np.ascontiguousarray(bq[hs:hs + CS][None, :]),
            "bkv": np.ascontiguousarray(bk[hs:hs + CS][None, :]),
            "bvv": np.ascontiguousarray(bv[hs:hs + CS][None, :]),
            "keep": keepm[b],
        })

    res = run_bass_kernel_spmd(nc, in_maps, list(range(N_CORES)), trace=TRACE)
    LAST_EXEC_NS = res.exec_time_ns

    outp = np.zeros((B, N, D), np.float32)
    for c in range(N_CORES):
        outp[c // 4] += res.results[c]["out"]
    outp += bo
    return outp



# revision 4
# speedup vs baseline: 1.4331x; 1.4331x over previous
"""Multi-head cross-attention (B=2, N=1024, L=4096, D=1024, H=16) on 8 trn2
NeuronCores — bf16 v2.

Sharding: batch x head-group data/tensor parallel. Core c handles batch
c//4 and heads 4*(c%4) .. 4*(c%4)+3 (weight columns sliced per head group,
Wo row-sliced; partial outputs summed on the host during unsharding).

v2 changes vs the fp32r baseline:
  - all matmul operands bf16 (fp32 PSUM accumulate): fp32 moving operands
    stream at 2 cycles/col on the PE xbus, bf16 at 1 — halves matmul time
    and DMA traffic, and the DMA'd bf16 tiles feed matmuls directly (no
    fp32->fp32r DVE casts).
  - padding mask applied as a per-key additive bias (-60) inside the exp
    activation (bias is a [128,1] per-partition AP), so V needs no keep
    premultiply; the augmented-V ones column provides the denominator.
  - q/k biases folded into the PSUM->SBUF copies (tensor_scalar_add with a
    per-partition bias vector); v bias added during the va build.
  - AV matmuls for key-block kb are dripped into kb+1's projection matmuls
    so the PE never stalls on the exp (ACT) latency.
"""
import sys

sys.path.insert(0, "/opt/trn_rl_repo")

import numpy as np

import concourse.bass as bass
import concourse.tile as tile
from concourse import bacc, mybir
from concourse.bass_utils import run_bass_kernel_spmd

dt = mybir.dt
ts = bass.ts

B, N, L, D = 2, 1024, 4096, 1024
H, DH = 16, 64
HC = 4            # heads per core
CS = HC * DH      # 256 channel slice per core
SCALE = DH ** -0.5
N_CORES = 8
QB, KB = 2, 8     # query blocks of 512, key blocks of 512
DQC = 8           # contraction chunks of 128
KT = 32           # keytiles of 128
MASK_BIAS = -60.0

TRACE = False
LAST_EXEC_NS = None
_cache = {}


def _build():
    nc = bacc.Bacc("TRN2", target_bir_lowering=False, debug=False,
                   num_devices=N_CORES)
    bf = dt.bfloat16

    xTq = nc.dram_tensor("xTq", [D, N], bf, kind="ExternalInput").ap()
    xTkv = nc.dram_tensor("xTkv", [D, L], bf, kind="ExternalInput").ap()
    wq = nc.dram_tensor("wq", [D, CS], bf, kind="ExternalInput").ap()
    wk = nc.dram_tensor("wk", [D, CS], bf, kind="ExternalInput").ap()
    wv = nc.dram_tensor("wv", [D, CS], bf, kind="ExternalInput").ap()
    wo = nc.dram_tensor("wo", [CS, D], bf, kind="ExternalInput").ap()
    bq2 = nc.dram_tensor("bq2", [128, 2], dt.float32, kind="ExternalInput").ap()
    bk2 = nc.dram_tensor("bk2", [128, 2], dt.float32, kind="ExternalInput").ap()
    bvb = nc.dram_tensor("bvb", [128, CS], dt.float32, kind="ExternalInput").ap()
    mb = nc.dram_tensor("mb", [128, KT], dt.float32, kind="ExternalInput").ap()
    out = nc.dram_tensor("out", [N, D], bf, kind="ExternalOutput").ap()

    with tile.TileContext(nc) as tc:
        _emit(nc, tc, xTq, xTkv, wq, wk, wv, wo, bq2, bk2, bvb, mb, out)
    nc.compile()
    return nc


def _emit(nc, tc, xTq, xTkv, wq, wk, wv, wo, bq2, bk2, bvb, mb, out):
    import contextlib

    bf = dt.bfloat16
    f32 = dt.float32
    ctx = contextlib.ExitStack()
    with ctx:
        persist = ctx.enter_context(tc.tile_pool(name="persist", bufs=1))
        xpool = ctx.enter_context(tc.tile_pool(name="xs", bufs=12))
        pT_pool = ctx.enter_context(tc.tile_pool(name="pT", bufs=10))
        rb_pool = ctx.enter_context(tc.tile_pool(name="rbs", bufs=2))
        outsb_pool = ctx.enter_context(tc.tile_pool(name="outsb", bufs=2))
        psT = ctx.enter_context(tc.tile_pool(name="psT", bufs=2, space="PSUM"))
        psOA_cm = tc.tile_pool(name="psOA", bufs=1, space="PSUM")
        psOA = psOA_cm.__enter__()
        lp = nc.allow_low_precision(reason="bf16 attention internals")
        lp.__enter__()

        def load_w3(name, src, d0):
            # src: DRAM [d0*128, F] bf16; dst tile [128, d0, F] (chunk-major)
            r = persist.tile([128, d0, src.shape[1]], bf, tag=name, name=name)
            nc.sync.dma_start(r[:], src.rearrange("(c p) n -> p c n", p=128))
            return r

        # ---- weights needed for the Q projection ------------------------
        wq_r = load_w3("wqr", wq, DQC)          # [128, 8, 256]
        bq_v = persist.tile([128, 2], f32, tag="bqv", name="bq_v")
        nc.sync.dma_start(bq_v[:], bq2)
        mb_t = persist.tile([128, KT], f32, tag="mbt", name="mb_t")
        nc.sync.dma_start(mb_t[:], mb)

        # ---- persistent activation tiles --------------------------------
        qT_sb = [persist.tile([128, N], bf, tag=f"qT{cc}", name=f"qT{cc}")
                 for cc in range(2)]
        kT_sb = [[persist.tile([128, 512], bf, tag=f"kT{cc}_{kb}",
                               name=f"kT{cc}_{kb}") for kb in range(KB)]
                 for cc in range(2)]
        va_sb = [persist.tile([128, HC, 65], bf, tag=f"va{kt}",
                              name=f"va{kt}") for kt in range(KT)]
        onT_sb = [persist.tile([128, N], bf, tag=f"onT{cc}",
                               name=f"onT{cc}") for cc in range(2)]

        # ---- Q projection ----------------------------------------------
        for qb in range(QB):
            qp = psT.tile([128, 1024], f32, tag="pp", name=f"qp{qb}")
            for dq in range(DQC):
                xf = xpool.tile([128, 512], bf, tag="xs", name=f"xfq{qb}_{dq}")
                nc.sync.dma_start(xf[:], xTq[ts(dq, 128), ts(qb, 512)])
                for cc in range(2):
                    nc.tensor.matmul(qp[:, ts(cc, 512)], wq_r[:, dq, ts(cc, 128)],
                                     xf[:], start=(dq == 0), stop=(dq == DQC - 1))
            for cc in range(2):
                nc.vector.tensor_scalar_add(qT_sb[cc][:, ts(qb, 512)],
                                            qp[:, ts(cc, 512)], bq_v[:, cc:cc + 1])

        # remaining weights (DMA priority after the q-projection inputs)
        wk_r = load_w3("wkr", wk, DQC)
        wv_r = load_w3("wvr", wv, DQC)
        wo_r = load_w3("wor", wo, 2)            # [128, 2, 1024]
        bk_v = persist.tile([128, 2], f32, tag="bkv", name="bk_v")
        nc.sync.dma_start(bk_v[:], bk2)
        bv_b = persist.tile([128, CS], f32, tag="bvb", name="bv_b")
        nc.sync.dma_start(bv_b[:], bvb)
        bv_b3 = bv_b[:].rearrange("p (h c) -> p h c", h=HC)
        ones128 = persist.tile([1, 128], bf, tag="o128", name="ones128")
        nc.vector.memset(ones128[:], 1.0)
        # augmented-V ones column (denominator row), set once
        for kt in range(KT):
            nc.vector.memset(va_sb[kt][:, :, 64:65], 1.0)

        # ---- attention helpers ------------------------------------------
        oPs = {}

        def open_oP(qb, hp, pool, sfx):
            oPs[(qb, hp)] = [
                pool.tile([128, 512], f32, tag=f"oP{qb}{hp}{h}{sfx}",
                          name=f"oP{qb}{hp}{h}{sfx}")
                for h in range(2)
            ]

        def attn_qk(qb, hp, kt):
            kb, kti = kt // 4, kt % 4
            sp = psT.tile([128, 1024], f32, tag="pp", name=f"sp{qb}{hp}{kt}")
            for h in range(2):
                nc.tensor.matmul(
                    sp[:, ts(h, 512)],
                    kT_sb[hp][kb][ts(h, 64), ts(kti, 128)],
                    qT_sb[hp][ts(h, 64), ts(qb, 512)],
                    start=True, stop=True,
                )
            pT = pT_pool.tile([128, 1024], bf, tag="pT", name=f"pT{qb}{hp}{kt}")
            nc.scalar.activation(pT[:], sp[:], mybir.ActivationFunctionType.Exp,
                                 scale=float(SCALE), bias=mb_t[:, kt:kt + 1])
            return pT

        def attn_av(qb, hp, kt, pT):
            oP = oPs[(qb, hp)]
            for h in range(2):
                nc.tensor.matmul(
                    oP[h][0:65, :], va_sb[kt][:, hp * 2 + h, :], pT[:, ts(h, 512)],
                    start=(kt == 0), stop=(kt == KT - 1),
                )

        def attn_norm(qb, hp):
            oP = oPs.pop((qb, hp))
            rb = psT.tile([128, 1024], f32, tag="pp", name=f"rb{qb}{hp}")
            rb_sb = rb_pool.tile([128, 1024], f32, tag="rbs", name=f"rbs{qb}{hp}")
            for h in range(2):
                den = rb_pool.tile([1, 512], f32, tag="den", name=f"den{qb}{hp}{h}")
                nc.vector.tensor_copy(den[:], oP[h][64:65, :])
                rdf = rb_pool.tile([1, 512], f32, tag="rdf", name=f"rdf{qb}{hp}{h}")
                # approx_fast needs an SBUF source (PSUM source returns garbage)
                nc.vector.reciprocal_approx_fast(rdf[:], den[:])
                rd = rb_pool.tile([1, 512], bf, tag="rd", name=f"rd{qb}{hp}{h}")
                nc.vector.tensor_copy(rd[:], rdf[:])
                nc.tensor.matmul(rb[:, ts(h, 512)], ones128[:], rd[:],
                                 start=True, stop=True)
            nc.vector.tensor_copy(rb_sb[:], rb[:])
            for h in range(2):
                nc.vector.tensor_mul(onT_sb[hp][ts(h, 64), ts(qb, 512)],
                                     oP[h][0:64, :], rb_sb[0:64, ts(h, 512)])

        # ---- phase A: K/V projections + attention on hp=0 (both qb) -----
        open_oP(0, 0, psOA, "a")
        open_oP(1, 0, psOA, "a")
        pend_av = []

        def drip():
            if pend_av:
                attn_av(*pend_av.pop(0))

        for kb in range(KB):
            kp = psT.tile([128, 1024], f32, tag="pp", name=f"kp{kb}")
            xks = []
            for dq in range(DQC):
                xf = xpool.tile([128, 512], bf, tag="xs", name=f"xfk{kb}_{dq}")
                nc.sync.dma_start(xf[:], xTkv[ts(dq, 128), ts(kb, 512)])
                xks.append(xf)
                for cc in range(2):
                    nc.tensor.matmul(kp[:, ts(cc, 512)], wk_r[:, dq, ts(cc, 128)],
                                     xf[:], start=(dq == 0), stop=(dq == DQC - 1))
                drip()
            for cc in range(2):
                nc.vector.tensor_scalar_add(kT_sb[cc][kb][:], kp[:, ts(cc, 512)],
                                            bk_v[:, cc:cc + 1])

            vp = psT.tile([128, 1024], f32, tag="pp", name=f"vp{kb}")
            for dq in range(DQC):
                for t in range(4):
                    # start clears has_written for the whole 2KB psum bank, so
                    # only the first matmul touching each bank may set it
                    nc.tensor.matmul(vp[:, ts(t, 256)], xks[dq][:, ts(t, 128)],
                                     wv_r[:, dq, :],
                                     start=(dq == 0 and t % 2 == 0),
                                     stop=(dq == DQC - 1 and t % 2 == 1))
                drip()
            for t in range(4):
                kt = kb * 4 + t
                src = vp[:, ts(t, 256)].rearrange("p (h c) -> p h c", h=HC)
                nc.vector.tensor_add(va_sb[kt][:, :, 0:64], src, bv_b3)

            for t in range(4):
                kt = kb * 4 + t
                for qb in range(QB):
                    pT = attn_qk(qb, 0, kt)
                    pend_av.append((qb, 0, kt, pT))

        while pend_av:
            drip()
        attn_norm(0, 0)
        attn_norm(1, 0)

        # ---- phase B: attention on hp=1 (both qb) -----------------------
        psOA_cm.__exit__(None, None, None)
        psOB_cm = tc.tile_pool(name="psOB", bufs=1, space="PSUM")
        psOB = psOB_cm.__enter__()
        open_oP(0, 1, psOB, "b")
        open_oP(1, 1, psOB, "b")
        for kt in range(KT):
            for qb in range(QB):
                pT = attn_qk(qb, 1, kt)
                pend_av.append((qb, 1, kt, pT))
            while len(pend_av) > 4:
                drip()
        while pend_av:
            drip()
        attn_norm(0, 1)
        attn_norm(1, 1)

        # ---- output projection ------------------------------------------
        for qt in range(8):
            op = psT.tile([128, 1024], f32, tag="pp", name=f"op{qt}")
            for eb in range(2):
                for cc in range(2):
                    nc.tensor.matmul(op[:, ts(eb, 512)], onT_sb[cc][:, ts(qt, 128)],
                                     wo_r[:, cc, ts(eb, 512)],
                                     start=(cc == 0), stop=(cc == 1))
            osb = outsb_pool.tile([128, 1024], bf, tag="osb", name=f"osb{qt}")
            nc.vector.tensor_copy(osb[:], op[:])
            nc.sync.dma_start(out[ts(qt, 128), :], osb[:])

        psOB_cm.__exit__(None, None, None)
        lp.__exit__(None, None, None)


def kernel(x_q, x_kv, pad_mask, Wq, bq, Wk, bk, Wv, bv, Wo, bo):
    global LAST_EXEC_NS
    import ml_dtypes
    bf16 = ml_dtypes.bfloat16

    x_q = np.asarray(x_q, np.float32)
    x_kv = np.asarray(x_kv, np.float32)
    pad_mask = np.asarray(pad_mask)
    Wq, bq = np.asarray(Wq, np.float32), np.asarray(bq, np.float32)
    Wk, bk = np.asarray(Wk, np.float32), np.asarray(bk, np.float32)
    Wv, bv = np.asarray(Wv, np.float32), np.asarray(bv, np.float32)
    Wo, bo = np.asarray(Wo, np.float32), np.asarray(bo, np.float32)

    if "nc" not in _cache:
        _cache["nc"] = _build()
    nc = _cache["nc"]

    xTq_b = [np.ascontiguousarray(x_q[b].T.astype(bf16)) for b in range(B)]
    xTkv_b = [np.ascontiguousarray(x_kv[b].T.astype(bf16)) for b in range(B)]
    mb_b = []
    for b in range(B):
        m = np.where(pad_mask[b], np.float32(MASK_BIAS), np.float32(0.0))
        mb_b.append(np.ascontiguousarray(m.reshape(KT, 128).T.astype(np.float32)))

    in_maps = []
    for c in range(N_CORES):
        b, g = c // 4, c % 4
        hs = g * CS
        in_maps.append({
            "xTq": xTq_b[b],
            "xTkv": xTkv_b[b],
            "wq": np.ascontiguousarray(Wq[:, hs:hs + CS].astype(bf16)),
            "wk": np.ascontiguousarray(Wk[:, hs:hs + CS].astype(bf16)),
            "wv": np.ascontiguousarray(Wv[:, hs:hs + CS].astype(bf16)),
            "wo": np.ascontiguousarray(Wo[hs:hs + CS, :].astype(bf16)),
            "bq2": np.ascontiguousarray(bq[hs:hs + CS].reshape(2, 128).T),
            "bk2": np.ascontiguousarray(bk[hs:hs + CS].reshape(2, 128).T),
            "bvb": np.ascontiguousarray(
                np.broadcast_to(bv[hs:hs + CS], (128, CS)).astype(np.float32)),
            "mb": mb_b[b],
        })

    res = run_bass_kernel_spmd(nc, in_maps, list(range(N_CORES)), trace=TRACE)
    LAST_EXEC_NS = res.exec_time_ns

    outp = np.zeros((B, N, D), np.float32)
    for c in range(N_CORES):
        outp[c // 4] += res.results[c]["out"].astype(np.float32)
    outp += bo
    return outp
